# revision 1
# baseline (speedup 1.0000x reference)
"""Trainium2 Bass kernel for nn_LogicConstraintLoss.

Contract: kernel(**inputs) takes FULL inputs, returns FULL output [3] f32
  (sym, trans, excl).

Math (verified vs reference):
  - The reference's torch-faithful scatter makes triplet_mask nonzero only at
    j == 0, so the N^3 transitivity term collapses to an O(N^2) computation
    using column 0 / row 0 of each transitive channel.
  - clip(x, 0) inside the violation is redundant because probs >= 0:
    relu(relu(a) - b) == relu(a - b) for b >= 0.
  - The triplet mask folds into an affine term: mask * relu(x) ==
    relu(x + 2*mask - 2) for x <= 1 (true here: x = ci + rk - 1 - rel <= 1).
  - Host pre-multiplies relation_probs by the pair mask (for the all-ones
    node_mask this is just zeroing the diagonal), which removes every other
    mask from the device program. The per-partition column term colr and all
    mask/affine constants are folded into the host-built rbt tensor.

Sharding: core c owns i-rows [40c, 40c+40) of both batches -> 80 partitions.
Per-core device inputs (host-prepped, contiguous):
  rs  [80,1920] f32 : row slice, free = (j, channel) interleaved
  ct  [80, 640] f32 : transposed col slice, channels 4,5: ct[(b,i'),(j,u)]
                      = rp[b, j, 40c+i', 4+u]
  rbt [80, 640] f32 : rbt[(b,i'),(k,ri)] = row_r[b,k] + 2*tm[b,i,k] - 3
                      + col_r[b,i],  r = (0,2)[ri]
Device: 3 wide fused ops per j-chunk (sym sub, excl paired stt, trans sub)
plus 2 ACT accumulations; emits per-partition partials in out[80, 4*nj].
"""

import numpy as np

B, N, R, K = 2, 320, 6, 16
NCORES = 8
S = N // NCORES          # 40 i-rows per core
P = B * S                # 80 partitions
TRANSITIVE = (0, 2)

NJ = 2                   # j-chunks for DMA/compute overlap
EXCL_ENGINE = "gpsimd"   # which engine runs the excl product stt
_PROGRAM = None


def _build_program(nj=NJ, excl_engine=EXCL_ENGINE):
    import concourse.bacc as bacc
    import concourse.mybir as mybir
    from concourse.tile import TileContext

    f32 = mybir.dt.float32
    nc = bacc.Bacc("TRN2", target_bir_lowering=False, debug=False)

    rs_d = nc.dram_tensor("rs", [P, N * R], f32, kind="ExternalInput")
    ct_d = nc.dram_tensor("ct", [P, N * 2], f32, kind="ExternalInput")
    rbt_d = nc.dram_tensor("rbt", [P, N * 2], f32, kind="ExternalInput")
    ncol = 4 * nj
    out_d = nc.dram_tensor("out", [P, ncol], f32, kind="ExternalOutput")

    jc = N // nj          # j per chunk
    rs3d = rs_d[:].rearrange("p (j c) -> p j c", c=R)
    ct3d = ct_d[:].rearrange("p (j u) -> p j u", u=2)
    rbt3d = rbt_d[:].rearrange("p (j u) -> p j u", u=2)

    with TileContext(nc) as tc:
        with tc.tile_pool(name="pool", bufs=1) as pool:
            OUT = pool.tile([P, ncol], f32)
            nc.vector.memset(OUT[:], 0.0)
            # spread chunk DMAs over distinct sequencers -> parallel DGE queues
            dma_engines = [nc.sync, nc.scalar, nc.gpsimd]
            di = 0
            for k in range(nj):
                j0 = k * jc
                RS = pool.tile([P, jc * R], f32, tag=f"rs{k}")
                CT = pool.tile([P, jc * 2], f32, tag=f"ct{k}")
                RBT = pool.tile([P, jc * 2], f32, tag=f"rbt{k}")
                MX = pool.tile([P, jc * 2], f32, tag=f"mx{k}")
                MN = pool.tile([P, jc * 2], f32, tag=f"mn{k}")
                W = pool.tile([P, jc * 2], f32, tag=f"w{k}")
                V = pool.tile([P, jc * 2], f32, tag=f"v{k}")
                V2 = pool.tile([P, jc * 2], f32, tag=f"v2{k}")

                for dst, src in ((RS[:], rs3d[:, j0:j0 + jc, :]),
                                 (CT[:], ct3d[:, j0:j0 + jc, :]),
                                 (RBT[:], rbt3d[:, j0:j0 + jc, :])):
                    dma_engines[di % len(dma_engines)].dma_start(out=dst, in_=src)
                    di += 1

                # channel views: rs4[p, j, pair, two], channel = pair*2 + two
                rs4 = RS[:].rearrange("p (j pr two) -> p j pr two", pr=3, two=2)
                ct4 = CT[:].rearrange("p (j one u) -> p j one u", one=1, u=2)
                mx4 = MX[:].rearrange("p (j one u) -> p j one u", one=1, u=2)
                mn4 = MN[:].rearrange("p (j one u) -> p j one u", one=1, u=2)
                w4 = W[:].rearrange("p (j pr one) -> p j pr one", pr=2, one=1)
                v4 = V[:].rearrange("p (j pr one) -> p j pr one", pr=2, one=1)
                rbt4 = RBT[:].rearrange("p (j pr one) -> p j pr one", pr=2, one=1)

                # ---- sym: |rs45 - ct| summed (sub on DVE, abs+accum on ACT) ----
                nc.vector.tensor_sub(mx4, rs4[:, :, 2:3, :], ct4)
                nc.scalar.activation(
                    out=MN[:], in_=MX[:],
                    func=mybir.ActivationFunctionType.Abs,
                    accum_out=OUT[:, 4 * k:4 * k + 1],
                )

                # ---- excl: p0*p1 + p2*p3 in one paired stt ----
                nc.vector.scalar_tensor_tensor(
                    out=w4,
                    in0=rs4[:, :, 0:2, 0:1],
                    scalar=0.0,
                    in1=rs4[:, :, 0:2, 1:2],
                    op0=mybir.AluOpType.bypass,
                    op1=mybir.AluOpType.mult,
                    accum_out=OUT[:, 4 * k + 1:4 * k + 2],
                )

                # ---- trans: relu(rbt - rel_{0,2}) summed (both r together) ----
                nc.vector.tensor_sub(v4, rbt4, rs4[:, :, 0:2, 0:1])
                nc.scalar.activation(
                    out=V2[:], in_=V[:], func=mybir.ActivationFunctionType.Relu,
                    accum_out=OUT[:, 4 * k + 2:4 * k + 3],
                )

            nc.sync.dma_start(out=out_d[:], in_=OUT[:])

    nc.compile()
    return nc


def _get_program():
    global _PROGRAM
    if _PROGRAM is None:
        _PROGRAM = _build_program()
    return _PROGRAM


def _host_prep(relation_probs, node_mask, knn_indices):
    """Build per-core input maps + host-side scalars (denom, count)."""
    rp = np.ascontiguousarray(np.asarray(relation_probs, dtype=np.float32))
    nm = np.asarray(node_mask, dtype=bool)
    knn = np.asarray(knn_indices)

    ar = np.arange(N)
    eye = ar[:, None] == ar[None, :]
    pm = nm[:, :, None] & nm[:, None, :] & ~eye[None]          # [B,N,N]
    denom = max(int(pm.sum()), 1)

    # trans mask tm[b,i,k]
    sampled = np.zeros((B, N, N), dtype=bool)
    bi = np.arange(B)[:, None, None]
    ii = ar[None, :, None]
    sampled[bi, ii, knn] = True
    i_ne0 = ar != 0
    tm = (nm[:, :, None] & nm[:, None, :] & nm[:, 0][:, None, None]
          & i_ne0[None, :, None] & i_ne0[None, None, :] & ~eye[None]) & sampled
    cnt = int(tm.sum())
    count = 2 * max(cnt, 1)

    # pre-mask rp by pm (all-ones node_mask: just zero the diagonal)
    if nm.all():
        rpm = rp.copy()
        rpm[:, ar, ar, :] = 0.0
    else:
        rpm = rp * pm[..., None].astype(np.float32)

    tmf = tm.astype(np.float32)
    row = rpm[:, 0, :, :]                                       # [B,N,R]
    col = rpm[:, :, 0, :]                                       # [B,N,R]

    in_maps = []
    for c in range(NCORES):
        sl = slice(c * S, (c + 1) * S)
        rs = np.ascontiguousarray(rpm[:, sl, :, :]).reshape(P, N * R)
        ct = np.ascontiguousarray(
            np.swapaxes(rpm[:, :, sl, 4:6], 1, 2)).reshape(P, N * 2)
        rbt = np.empty((B, S, N, 2), dtype=np.float32)
        t2 = 2.0 * tmf[:, sl, :] - 3.0                          # [B,S,N]
        for ri, r in enumerate(TRANSITIVE):
            rbt[:, :, :, ri] = (row[:, None, :, r] + t2
                                + col[:, sl, None, r])
        in_maps.append({
            "rs": rs,
            "ct": ct,
            "rbt": np.ascontiguousarray(rbt).reshape(P, N * 2),
        })
    return in_maps, denom, count


def kernel(relation_probs, node_mask, knn_indices):
    from concourse.bass_utils import run_bass_kernel_spmd

    in_maps, denom, count = _host_prep(relation_probs, node_mask, knn_indices)
    nc = _get_program()
    res = run_bass_kernel_spmd(nc, in_maps, core_ids=list(range(NCORES)))

    sym_sum = 0.0
    ex = 0.0
    tr = 0.0
    for om in res.results:
        o = om["out"].astype(np.float64)
        for k in range(NJ):
            sym_sum += o[:, 4 * k].sum()
            ex += o[:, 4 * k + 1].sum()
            tr += o[:, 4 * k + 2].sum()

    sym = sym_sum / denom
    trans = tr / count
    excl = ex / denom / 2.0
    return np.array([sym, trans, excl], dtype=np.float32)



# revision 4
# speedup vs baseline: 1.2197x; 1.2197x over previous
"""Trainium2 Bass kernel for nn_LogicConstraintLoss.

Contract: kernel(**inputs) takes FULL inputs, returns FULL output [3] f32
  (sym, trans, excl).

Math (verified vs reference):
  - The reference's torch-faithful scatter makes triplet_mask nonzero only at
    j == 0, so the N^3 transitivity term collapses to an O(N^2) computation;
    additionally only the <=K sampled k per (b,i) row survive the mask, so the
    device consumes a gathered [rows, K] stream of (premise-affine, rel) pairs
    and evaluates relu(t - v) over it.
  - clip(x, 0) inside the violation is redundant because probs >= 0.
  - sym: |p_ij - p_ji| summed over ordered pairs == 2 * sum over unordered
    pairs, so each off-diagonal element ships exactly once (halves sym bytes).
  - excl: p0*p1 + p2*p3 as one elementwise product of two channel-interleaved
    streams.

Device layout: every stream is flattened and reshaped to [128, F] so all 128
SBUF partitions (and all 16 SDMA engines) are used; compute is elementwise +
free-dim accumulate, so partition boundaries need not align with rows.
All streams ship as bf16 (tolerance is 2e-2; measured error ~1e-3).

Per-core inputs:
  d1 [128, 440] bf16 : [sa 200 | sb 200 | tt 20 | tv 20]
  d2 [128, 800] bf16 : [ea 400 | eb 400]
Device: DVE sub+abs (sym), sub+relu (trans), product (excl), each with an
f32 accum column; PE matmul with a ones vector reduces the [128,3] partials
across partitions so the output DMA is a single [1,3] descriptor.
"""

import numpy as np

B, N, R, K = 2, 320, 6, 16
NCORES = 8
P = 128
SYM_F = 200              # sym pair slots per partition (each half)
TT_F = 20                # trans slots per partition (B*N*K*2 / NCORES / 128)
EX_F = 400               # excl cols per partition (each half)
D1_F = 2 * SYM_F + 2 * TT_F
D2_F = 2 * EX_F

_PROGRAM = None


def _build_program():
    import concourse.bass as bass
    import concourse.bacc as bacc
    import concourse.mybir as mybir
    from concourse.tile import TileContext

    f32 = mybir.dt.float32
    bf16 = mybir.dt.bfloat16
    nc = bacc.Bacc("TRN2", target_bir_lowering=False, debug=False)

    d1_d = nc.dram_tensor("d1", [P, D1_F], bf16, kind="ExternalInput")
    d2_d = nc.dram_tensor("d2", [P, D2_F], bf16, kind="ExternalInput")
    out_d = nc.dram_tensor("out", [1, 3], f32, kind="ExternalOutput")

    with TileContext(nc) as tc:
        with (
            tc.tile_pool(name="pool", bufs=1) as pool,
            tc.tile_pool(name="psum", bufs=1, space=bass.MemorySpace.PSUM) as pp,
        ):
            D1 = pool.tile([P, D1_F], bf16)
            D2 = pool.tile([P, D2_F], bf16)
            WS = pool.tile([P, SYM_F], bf16)
            WSA = pool.tile([P, SYM_F], bf16)
            WT = pool.tile([P, TT_F], bf16)
            WTA = pool.tile([P, TT_F], bf16)
            WE = pool.tile([P, EX_F], bf16)
            ACC = pool.tile([P, 3], f32)
            ONES = pool.tile([P, 1], f32)
            OUTS = pool.tile([1, 3], f32)
            PS = pp.tile([1, 3], f32)

            # two parallel HWDGE queues; PL memsets the matmul ones-vector
            nc.sync.dma_start(out=D1[:], in_=d1_d[:])
            nc.scalar.dma_start(out=D2[:], in_=d2_d[:])
            nc.gpsimd.memset(ONES[:], 1.0)

            mx = mybir.AluOpType.max
            # sym: |sa - sb|, accumulate
            nc.vector.tensor_sub(WS[:], D1[:, 0:SYM_F], D1[:, SYM_F:2 * SYM_F])
            nc.vector.scalar_tensor_tensor(
                out=WSA[:], in0=WS[:], scalar=-1.0, in1=WS[:],
                op0=mybir.AluOpType.mult, op1=mx, accum_out=ACC[:, 0:1])
            # trans: relu(tt - tv), accumulate
            t0 = 2 * SYM_F
            nc.vector.tensor_sub(WT[:], D1[:, t0:t0 + TT_F],
                                 D1[:, t0 + TT_F:t0 + 2 * TT_F])
            nc.vector.tensor_scalar(
                out=WTA[:], in0=WT[:], scalar1=0.0, scalar2=0.0,
                op0=mx, op1=mybir.AluOpType.add, accum_out=ACC[:, 1:2])
            # excl: ea * eb, accumulate
            nc.vector.scalar_tensor_tensor(
                out=WE[:], in0=D2[:, 0:EX_F], scalar=0.0, in1=D2[:, EX_F:2 * EX_F],
                op0=mybir.AluOpType.bypass, op1=mybir.AluOpType.mult,
                accum_out=ACC[:, 2:3])

            # reduce partials across partitions: ones[128,1].T @ ACC[128,3]
            nc.tensor.matmul(PS[:], ONES[:], ACC[:])
            nc.vector.tensor_copy(OUTS[:], PS[:])
            nc.sync.dma_start(out=out_d[:], in_=OUTS[:])

    nc.compile()
    return nc


def _get_program():
    global _PROGRAM
    if _PROGRAM is None:
        _PROGRAM = _build_program()
    return _PROGRAM


def _host_prep(relation_probs, node_mask, knn_indices):
    """Marshal inputs into per-core [128, F] bf16 streams."""
    import ml_dtypes

    rp = np.ascontiguousarray(np.asarray(relation_probs, dtype=np.float32))
    nm = np.asarray(node_mask, dtype=bool)
    knn = np.asarray(knn_indices)

    ar = np.arange(N)
    eye = ar[:, None] == ar[None, :]
    pm = nm[:, :, None] & nm[:, None, :] & ~eye[None]          # [B,N,N]
    denom = max(int(pm.sum()), 1)

    if nm.all():
        rpm = rp.copy()
        rpm[:, ar, ar, :] = 0.0
    else:
        rpm = rp * pm[..., None].astype(np.float32)

    # ---- excl streams: channels (0,2) x (1,3) ----
    ea = np.ascontiguousarray(rpm[..., [0, 2]]).reshape(NCORES, P, EX_F)
    eb = np.ascontiguousarray(rpm[..., [1, 3]]).reshape(NCORES, P, EX_F)

    # ---- sym pair streams: each unordered off-diag pair shipped once ----
    iu, ju = np.triu_indices(N, 1)
    sa = np.ascontiguousarray(rpm[:, iu, ju][..., [4, 5]]).reshape(-1)
    sb = np.ascontiguousarray(rpm[:, ju, iu][..., [4, 5]]).reshape(-1)
    pad = NCORES * P * SYM_F - sa.size
    assert pad >= 0
    sa = np.concatenate([sa, np.zeros(pad, np.float32)]).reshape(NCORES, P, SYM_F)
    sb = np.concatenate([sb, np.zeros(pad, np.float32)]).reshape(NCORES, P, SYM_F)

    # ---- trans sampled-triplet streams ----
    sampled = np.zeros((B, N, N), dtype=bool)
    bi = np.arange(B)[:, None, None]
    ii = ar[None, :, None]
    sampled[bi, ii, knn] = True
    i_ne0 = ar != 0
    tm = (nm[:, :, None] & nm[:, None, :] & nm[:, 0][:, None, None]
          & i_ne0[None, :, None] & i_ne0[None, None, :] & ~eye[None]) & sampled
    cnt = int(tm.sum())
    count = 2 * max(cnt, 1)

    # pads keep t - v = -1 -> relu contributes 0
    tarr = np.full((B, N, K, 2), -1.0, dtype=np.float32)
    varr = np.zeros((B, N, K, 2), dtype=np.float32)
    bb, ii2, kk = np.nonzero(tm)
    if len(bb):
        key = bb * N + ii2                       # nondecreasing (row-major)
        first = np.r_[0, np.flatnonzero(np.diff(key)) + 1]
        counts = np.diff(np.r_[first, len(bb)])
        slot = np.arange(len(bb)) - np.repeat(first, counts)
        assert slot.max() < K
        row0 = rp[:, 0, :, :]                    # [B,N,R] raw row 0
        col0 = rp[:, :, 0, :]                    # [B,N,R] raw col 0
        for ri, r in enumerate((0, 2)):
            tarr[bb, ii2, slot, ri] = col0[bb, ii2, r] + row0[bb, kk, r] - 1.0
            varr[bb, ii2, slot, ri] = rp[bb, ii2, kk, r]
    t8 = tarr.reshape(NCORES, P, TT_F)
    v8 = varr.reshape(NCORES, P, TT_F)

    bf = ml_dtypes.bfloat16
    in_maps = []
    for c in range(NCORES):
        d1 = np.ascontiguousarray(
            np.concatenate([sa[c], sb[c], t8[c], v8[c]], axis=1).astype(bf))
        d2 = np.ascontiguousarray(
            np.concatenate([ea[c], eb[c]], axis=1).astype(bf))
        in_maps.append({"d1": d1, "d2": d2})
    return in_maps, denom, count


def kernel(relation_probs, node_mask, knn_indices):
    from concourse.bass_utils import run_bass_kernel_spmd

    in_maps, denom, count = _host_prep(relation_probs, node_mask, knn_indices)
    nc = _get_program()
    res = run_bass_kernel_spmd(nc, in_maps, core_ids=list(range(NCORES)))

    sym_sum = 0.0
    tr = 0.0
    ex = 0.0
    for om in res.results:
        o = om["out"].astype(np.float64).reshape(-1)
        sym_sum += o[0]
        tr += o[1]
        ex += o[2]

    sym = 2.0 * sym_sum / denom
    trans = tr / count
    excl = ex / denom / 2.0
    return np.array([sym, trans, excl], dtype=np.float32)
